# revision 1
# baseline (speedup 1.0000x reference)
"""BitfieldLinear (vq_codebook) Trainium2 kernel.

y = x @ W^T + bias, W decoded from a VQ codebook:
    idx = codes & 0xFF            (basis row, 256 entries)
    r   = ((codes >> 8) & 0xFFFF) / 65535
    W   = r[:,None] * basis[idx] + scales[:,None] * (resid - 128) / 127

Sharding across 8 NeuronCores: 4-way over out_features (1024 each) x
2-way over flattened tokens (4096 each). Per core:
  - residual term decoded on ACT ([o,i] tiles), staged to DRAM, and
    xbar-transposed into W^T tiles [128 i_lo, 32 k, 512 o].
  - basis term computed directly in W^T layout by the tensor engine:
    baseT[i, o] = sum_b basis[b, i] * onehot[b, o], onehot = r[o] at
    row idx[o]; accumulated into the W^T tiles in place (DVE add).
  - x row-blocks stream via SWDGE cast-DMA (fp32->bf16) and xbar
    transpose; 64 bf16 matmuls per block accumulate K=4096 in PSUM;
    bias (ones (x) bias broadcast) is added during PSUM evacuation.
Host only slices inputs and reassembles the output.
"""

import numpy as np

import concourse.bass as bass
import concourse.mybir as mybir
import concourse.tile as tile
from concourse.masks import make_identity
from concourse.bass_utils import run_bass_kernel_spmd

# problem shape (hardcoded per harness contract)
B, S, D_IN, D_OUT, BASIS = 4, 2048, 4096, 4096, 256
N_CORES = 8
O_SHARDS, N_SHARDS = 4, 2           # grid: core = oc * N_SHARDS + nb
O_SH = D_OUT // O_SHARDS            # 1024 out-features per core
N_SH = (B * S) // N_SHARDS          # 4096 token rows per core

P = 128
KC = D_IN // P                      # 32 contraction chunks
KH = KC // 2
OT = O_SH // P                      # 8 o-tiles per core
NOS = O_SH // 512                   # 2 PSUM o-slices per core
NB = N_SH // P                      # 32 token blocks per core
HALF = D_IN // 2

F32 = mybir.dt.float32
BF16 = mybir.dt.bfloat16
I32 = mybir.dt.int32

_WAIT_LIMIT = 1


def _split_sync_waits(nc):
    """walrus in this container rejects instructions with more than one
    embedded sync-wait command; hoist the excess onto same-engine NoOps."""
    ctr = 0
    for f in nc.m.functions:
        for bb in f.blocks:
            new = []
            changed = False
            for inst in bb.instructions:
                si = inst.sync_info
                if si is not None and si.on_wait and len(si.on_wait) > _WAIT_LIMIT:
                    waits = list(si.on_wait)
                    excess, keep = waits[:-_WAIT_LIMIT], waits[-_WAIT_LIMIT:]
                    for i in range(0, len(excess), _WAIT_LIMIT):
                        ctr += 1
                        new.append(mybir.InstNoOp(
                            name=f"I-waitsplit-{ctr}",
                            engine=inst.engine,
                            ins=[], outs=[],
                            sync_info=mybir.SyncInfo(
                                on_wait=excess[i:i + _WAIT_LIMIT], on_update=[]),
                        ))
                    si.on_wait = keep
                    changed = True
                new.append(inst)
            if changed:
                bb.instructions = new


def _build_program():
    nc = bass.Bass()
    Alu = mybir.AluOpType
    Act = mybir.ActivationFunctionType

    x_in = nc.dram_tensor("x_sh", [N_SH, D_IN], F32, kind="ExternalInput")
    codes_in = nc.dram_tensor("codes_sh", [O_SH], I32, kind="ExternalInput")
    basis_in = nc.dram_tensor("basis", [BASIS, D_IN], F32, kind="ExternalInput")
    resid_in = nc.dram_tensor("resid_sh", [O_SH, D_IN], I32, kind="ExternalInput")
    scales_in = nc.dram_tensor("scales_sh", [O_SH], F32, kind="ExternalInput")
    bias_in = nc.dram_tensor("bias_sh", [O_SH], F32, kind="ExternalInput")
    y_out = nc.dram_tensor("y_sh", [N_SH, O_SH], F32, kind="ExternalOutput")

    with tile.TileContext(nc) as tc:
        with (
            tc.tile_pool(name="const", bufs=1) as cpool,
            tc.tile_pool(name="dram", bufs=1, space="DRAM") as dpool,
            tc.tile_pool(name="resid", bufs=2) as rpool,
            tc.tile_pool(name="wnat", bufs=3) as wpool,
            tc.tile_pool(name="wt", bufs=1) as wtpool,
            tc.tile_pool(name="xnat", bufs=2) as xnpool,
            tc.tile_pool(name="xbf", bufs=2) as xpool,
            tc.tile_pool(name="xt", bufs=5) as xtpool,
            tc.tile_pool(name="y", bufs=2) as ypool,
            tc.tile_pool(name="psum", bufs=6, space="PSUM") as pspool,
        ):
            # ---- prep: decode scalars -------------------------------
            codes_row = wpool.tile([1, O_SH], I32, tag="w", name="codes_row")
            nc.sync.dma_start(codes_row[:], codes_in[None, :])
            tmp_row = wpool.tile([1, O_SH], I32, tag="w", name="tmp_row")
            nc.vector.tensor_scalar(tmp_row[:], codes_row[:], 0xFF, None,
                                    Alu.bitwise_and)
            idx_row_f = wpool.tile([1, O_SH], BF16, tag="w", name="idx_row_f")
            nc.scalar.activation(idx_row_f[:], tmp_row[:], Act.Copy)
            rq_row = wpool.tile([1, O_SH], I32, tag="w", name="rq_row")
            nc.vector.tensor_scalar(rq_row[:], codes_row[:], 8, 0xFFFF,
                                    Alu.logical_shift_right, Alu.bitwise_and)
            r_row_f = wpool.tile([1, O_SH], BF16, tag="w", name="r_row_f")
            nc.scalar.activation(r_row_f[:], rq_row[:], Act.Copy,
                                 scale=1.0 / 65535.0)

            s_pp = cpool.tile([P, OT], F32)
            nc.sync.dma_start(s_pp[:], scales_in.rearrange("(t p) -> p t", p=P))
            sv_pp = cpool.tile([P, OT], F32)
            nc.vector.tensor_scalar_mul(sv_pp[:], s_pp[:], 1.0 / 127.0)
            bv_pp = cpool.tile([P, OT], F32)
            nc.vector.tensor_scalar_mul(bv_pp[:], s_pp[:], -128.0 / 127.0)

            bias_row = cpool.tile([1, O_SH], BF16, name="bias_row")
            nc.gpsimd.dma_start(bias_row[:], bias_in[None, :])
            ones_row = cpool.tile([1, P], BF16)
            nc.vector.memset(ones_row[:], 1.0)

            # ---- basis table -> resident SBUF bf16 ------------------
            # basis tiles live in the xt pool: they occupy 2 of the 5
            # slots during the W build and free them for deeper x-runway
            # buffering once the last baseT matmul has read them.
            basis_sb = [xtpool.tile([P, D_IN], BF16, tag="xt",
                                    name=f"basis_sb{bh}")
                        for bh in range(2)]
            for bh in range(2):
                for hf in range(2):
                    bhs = slice(hf * HALF, (hf + 1) * HALF)
                    basis_f32 = rpool.tile([P, HALF], F32, tag="res",
                                           name=f"basisf{bh}_{hf}")
                    nc.gpsimd.dma_start(basis_f32[:], basis_in[bh * P:(bh + 1) * P, bhs])
                    nc.scalar.copy(basis_sb[bh][:, bhs], basis_f32[:])

            # ---- one-hot codebook selectors [128 b, O_SH o] ---------
            # onehot[bh][p, o] = r[o] if idx[o] == bh*128+p else 0
            iota_i = cpool.tile([P, 1], I32)
            nc.gpsimd.iota(iota_i[:], pattern=[[0, 1]], base=0,
                           channel_multiplier=1)
            iota_f = [cpool.tile([P, 1], F32, name=f"iota_f{bh}")
                      for bh in range(2)]
            nc.scalar.activation(iota_f[0][:], iota_i[:], Act.Copy)
            nc.scalar.activation(iota_f[1][:], iota_i[:], Act.Copy, bias=128.0,
                                 scale=1.0)
            onehot = [cpool.tile([P, O_SH], BF16, name=f"onehot{bh}")
                      for bh in range(2)]
            r_bc = cpool.tile([P, 512], BF16)
            for q in range(NOS):
                qs = slice(q * 512, (q + 1) * 512)
                pr = pspool.tile([P, 512], F32, tag="rt", bufs=2, name=f"pr{q}")
                nc.tensor.matmul(pr[:], lhsT=ones_row[:], rhs=r_row_f[:, qs],
                                 start=True, stop=True)
                nc.scalar.copy(r_bc[:], pr[:])
                pi = pspool.tile([P, 512], F32, tag="rt", bufs=2, name=f"pi{q}")
                nc.tensor.matmul(pi[:], lhsT=ones_row[:], rhs=idx_row_f[:, qs],
                                 start=True, stop=True)
                for bh in range(2):
                    # (idx - 128*bh == iota) * r
                    nc.vector.scalar_tensor_tensor(
                        onehot[bh][:, qs], pi[:], iota_f[bh][:, :1], r_bc[:],
                        op0=Alu.is_equal, op1=Alu.mult)

            # ---- bias broadcast [128, O_SH] -------------------------
            bias_bc = cpool.tile([P, O_SH], BF16)
            for q in range(NOS):
                qs = slice(q * 512, (q + 1) * 512)
                pb = pspool.tile([P, 512], F32, tag="rt", bufs=2, name=f"pb{q}")
                nc.tensor.matmul(pb[:], lhsT=ones_row[:], rhs=bias_row[:, qs],
                                 start=True, stop=True)
                nc.scalar.copy(bias_bc[:, qs], pb[:])

            # ---- W^T build ------------------------------------------
            # wt[os][p, k, o'] = W^T[i = k*128+p, o = os*512 + o']
            # residual term decoded [o, i] on ACT, transposed 128x128 on
            # the tensor engine (idle during build); basis term =
            # basis_sb.T @ onehot. Both land in PSUM; ACT/DVE combine.
            identity = cpool.tile([P, P], BF16)
            make_identity(nc, identity[:])
            wts = [wtpool.tile([P, KC, 512], BF16, tag=f"wt{os}", name=f"wt{os}")
                   for os in range(NOS)]
            for os in range(NOS):
                osl = slice(os * 512, (os + 1) * 512)
                for hf in range(2):
                    hs = slice(hf * HALF, (hf + 1) * HALF)
                    w_ths = []
                    for j in range(4):
                        t = os * 4 + j
                        resid_t = rpool.tile([P, HALF], I32, tag="res",
                                             name=f"res{t}_{hf}")
                        reng = nc.scalar if (t + hf) % 2 == 0 else nc.gpsimd
                        reng.dma_start(resid_t[:],
                                       resid_in[t * P:(t + 1) * P, hs])
                        # resid term: scales/127 * q - 128*scales/127
                        w_th = wpool.tile([P, HALF], BF16, tag="wth", bufs=6,
                                          name=f"w_{t}_{hf}")
                        nc.scalar.activation(w_th[:], resid_t[:], Act.Identity,
                                             bias=bv_pp[:, t:t + 1],
                                             scale=sv_pp[:, t:t + 1])
                        w_ths.append(w_th)
                    for kk in range(hf * KH, (hf + 1) * KH):
                        ks = slice((kk - hf * KH) * P, (kk - hf * KH + 1) * P)
                        prt = pspool.tile([P, 512], BF16, tag="rt", bufs=2,
                                          name=f"prt{os}_{kk}")
                        for j in range(4):
                            nc.tensor.transpose(prt[:, j * P:(j + 1) * P],
                                                w_ths[j][:, ks], identity[:])
                        pw = pspool.tile([P, 512], F32, tag="mm",
                                         name=f"pw{os}_{kk}")
                        for bh in range(2):
                            nc.tensor.matmul(pw[:],
                                             lhsT=basis_sb[bh][:, kk * P:(kk + 1) * P],
                                             rhs=onehot[bh][:, osl],
                                             start=(bh == 0), stop=(bh == 1))
                        nc.scalar.copy(wts[os][:, kk, :], prt[:])
                        nc.vector.tensor_add(wts[os][:, kk, :],
                                             wts[os][:, kk, :], pw[:])

            # ---- main loop: stream x blocks, matmul, evac -----------
            for nb in range(NB):
                x_bf = xpool.tile([P, D_IN], BF16, tag="xbf", name=f"xbf{nb}")
                for xh in range(2):
                    xs = slice(xh * HALF, (xh + 1) * HALF)
                    x_nat = xnpool.tile([P, HALF], F32, tag="xn",
                                        name=f"xn{nb}_{xh}")
                    xeng = nc.scalar if nb % 2 == 0 else nc.gpsimd
                    xeng.dma_start(x_nat[:], x_in[nb * P:(nb + 1) * P, xs])
                    nc.vector.tensor_copy(x_bf[:, xs], x_nat[:])
                xT = xtpool.tile([P, KC, P], BF16, tag="xt", name=f"xt{nb}")
                nc.sync.dma_start_transpose(xT[:], x_bf[:])

                y_sb = ypool.tile([P, O_SH], F32, tag="y", name=f"y{nb}")
                for os in range(NOS):
                    osl2 = slice(os * 512, (os + 1) * 512)
                    if nb < 2:
                        # split-K: each half-chain is schedulable as soon
                        # as that build quarter's W^T chunks exist, so
                        # these MMs fill the PE during the build ramp
                        psA = pspool.tile([P, 512], F32, tag="mm",
                                          name=f"psA{nb}_{os}")
                        for k in range(KH):
                            nc.tensor.matmul(psA[:], lhsT=xT[:, k, :],
                                             rhs=wts[os][:, k, :],
                                             start=(k == 0), stop=(k == KH - 1))
                        psB = pspool.tile([P, 512], F32, tag="mm",
                                          name=f"psB{nb}_{os}")
                        for k in range(KH, KC):
                            nc.tensor.matmul(psB[:], lhsT=xT[:, k, :],
                                             rhs=wts[os][:, k, :],
                                             start=(k == KH), stop=(k == KC - 1))
                        nc.vector.tensor_add(y_sb[:, osl2], psA[:],
                                             bias_bc[:, osl2])
                        nc.vector.tensor_add(y_sb[:, osl2], y_sb[:, osl2],
                                             psB[:])
                    else:
                        ps = pspool.tile([P, 512], F32, tag="mm",
                                         name=f"psmm{nb}_{os}")
                        for k in range(KC):
                            nc.tensor.matmul(ps[:], lhsT=xT[:, k, :],
                                             rhs=wts[os][:, k, :],
                                             start=(k == 0), stop=(k == KC - 1))
                        nc.vector.tensor_add(y_sb[:, osl2], ps[:],
                                             bias_bc[:, osl2])
                nc.scalar.dma_start(y_out[nb * P:(nb + 1) * P, :], y_sb[:])

    _split_sync_waits(nc)
    return nc


_program_cache = {}


def _get_program():
    if "nc" not in _program_cache:
        _program_cache["nc"] = _build_program()
    return _program_cache["nc"]


def kernel(x, codes, basis_table, residual_q, residual_scales, bias):
    x = np.ascontiguousarray(np.asarray(x, dtype=np.float32))
    codes = np.ascontiguousarray(np.asarray(codes, dtype=np.int32))
    basis_table = np.ascontiguousarray(np.asarray(basis_table, dtype=np.float32))
    residual_q = np.ascontiguousarray(np.asarray(residual_q, dtype=np.int32))
    residual_scales = np.ascontiguousarray(
        np.asarray(residual_scales, dtype=np.float32))
    bias = np.ascontiguousarray(np.asarray(bias, dtype=np.float32))

    x2 = x.reshape(B * S, D_IN)
    in_maps = []
    for core in range(N_CORES):
        oc, nb = divmod(core, N_SHARDS)
        osl = slice(oc * O_SH, (oc + 1) * O_SH)
        nsl = slice(nb * N_SH, (nb + 1) * N_SH)
        in_maps.append({
            "x_sh": np.ascontiguousarray(x2[nsl]),
            "codes_sh": np.ascontiguousarray(codes[osl]),
            "basis": basis_table,
            "resid_sh": np.ascontiguousarray(residual_q[osl]),
            "scales_sh": np.ascontiguousarray(residual_scales[osl]),
            "bias_sh": np.ascontiguousarray(bias[osl]),
        })

    nc = _get_program()
    res = run_bass_kernel_spmd(nc, in_maps, core_ids=list(range(N_CORES)))

    y = np.empty((B * S, D_OUT), dtype=np.float32)
    for core in range(N_CORES):
        oc, nb = divmod(core, N_SHARDS)
        y[nb * N_SH:(nb + 1) * N_SH, oc * O_SH:(oc + 1) * O_SH] = \
            res.results[core]["y_sh"]
    return y.reshape(B, S, D_OUT)

